# revision 48
# baseline (speedup 1.0000x reference)
"""Multi-head causal attention (B=4, T=2048, D=1024, H=16, hd=64) on 8 trn2 cores.

Sharding: core = (batch, head_group): 4 batches x 2 head-groups of 8 heads.
Each core computes its batch's attention for its 8 heads plus the partial
output projection; the host sums the two head-group partials per batch and
adds the output bias.

Per-core kernel (fp32 PSUM accumulation throughout):
  x and Wq/Wk/Wv live in fp8e4m3 (weights pre-scaled 32x on the host to
  escape the e4m3 subnormal range); Q/K/V projections run fp8 DoubleRow
  matmuls (two k-tiles contracted per pass).  The 32*32 score scale is
  undone inside the softmax exp scale; the 32x on V cancels against a 32.0
  denominator-ones column.
  per head-pair m, query chunk c (512 wide), j-tile t:
    S^T pair: two row-tiled bf16 matmuls (contract rows 0-63 / 64-127)
              issued back-to-back -> run concurrently on the PE, into one
              [128, 2x512] PSUM tile
    expS = exp(S^T / (8*1024))  one ACT instruction for both heads, ->fp8
    causal mask on the diagonal block (one GPSIMD affine_select, both heads)
    per t-PAIR: ctxT_aug += V_aug^T-contract expS, a single fp8 DoubleRow
              matmul contracting both j-tiles (row 64 = denominator)
  ctx = ctxT[0:64] * (1/denom)          (lazy DVE/GPSIMD chain, see below)
  out_partial = ctx^T-contract Wo_s     [2048, 1024] in bf16

The scalar-engine exp stream is the attention-phase bottleneck, so the
tensor engine is kept busy through its exp waits: the S matmuls run one
iteration ahead of the PV matmuls, and projection work (V tiles, the next
pair's Q/K projection, the output projection) is drip-fed between S and PV
one matmul at a time from a chunk-filler queue.  The normalize chains are
emitted lazily, one stage per iteration, so their cross-engine waits never
head-of-line block an engine FIFO.
"""

import os
import sys

sys.path.insert(0, "/opt/trn_rl_repo")

import numpy as np

B = 4
T = 2048
D = 1024
H = 16
HD = 64
NCORES = 8
HPC = 8          # heads per core
DPC = HPC * HD   # 512
KT = D // 128    # 8 k-tiles
NT = T // 128    # 16 token tiles

_CACHE = {}
LAST_RESULTS = None


def _build_program():
    from contextlib import ExitStack

    import concourse.bass as bass
    import concourse.tile as tile
    from concourse import bacc, mybir

    f32 = mybir.dt.float32
    bf16 = mybir.dt.bfloat16
    f8 = mybir.dt.float8e4
    DR = mybir.MatmulPerfMode.DoubleRow
    Exp = mybir.ActivationFunctionType.Exp

    nc = bacc.Bacc(
        "TRN2", target_bir_lowering=False, debug=False, num_devices=NCORES
    )
    # x and the projection weights are fp8 (weights pre-scaled by 32 on the
    # host so their ~N(0, 0.02) values escape the e4m3 subnormal range); the
    # 32*32 scale on the scores is undone in the softmax exp scale, and the
    # 32x on V cancels against a 32.0 denominator-ones column
    xT = nc.dram_tensor("xT", [D, T], f8, kind="ExternalInput").ap()
    wq_d = nc.dram_tensor("wq", [D, DPC], f8, kind="ExternalInput").ap()
    wk_d = nc.dram_tensor("wk", [D, DPC], f8, kind="ExternalInput").ap()
    wv_d = nc.dram_tensor("wv", [D, DPC], f8, kind="ExternalInput").ap()
    wo_d = nc.dram_tensor("wo", [DPC, D], bf16, kind="ExternalInput").ap()
    out_d = nc.dram_tensor("out", [T, D], f32, kind="ExternalOutput").ap()

    with tile.TileContext(nc) as tc, ExitStack() as top:
        persist = top.enter_context(tc.tile_pool(name="persist", bufs=1))
        xt_sb = persist.tile([128, KT, T], f8, tag="xt")
        qt = persist.tile([128, 4, T], bf16, tag="qt")
        kt = persist.tile([128, 4, T], bf16, tag="kt")
        # per-tile row stride 8*66=528 bytes keeps the t-pair step 16-aligned
        # for the DoubleRow PV weight AP; head h occupies cols 66h..66h+64
        v_sb = persist.tile([128, NT, HPC * (HD + 2)], f8, tag="v")
        ctx_sb = persist.tile([128, 4, T], bf16, tag="ctx")
        wq = persist.tile([128, KT, DPC], f8, tag="wq")
        wk = persist.tile([128, KT, DPC], f8, tag="wk")
        wv = persist.tile([128, KT, DPC], f8, tag="wv")
        wo = persist.tile([128, 4, D], bf16, tag="wo")

        # weight loads, one consolidated DMA each, on the gpsimd queue so
        # they overlap the x-chunk loads on the sync/scalar queues
        for w_sb, w_d in ((wv, wv_d), (wq, wq_d), (wk, wk_d)):
            nc.gpsimd.dma_start(
                out=w_sb, in_=w_d.rearrange("(k p) d -> p k d", p=128)
            )
        xT_r = xT.rearrange("(k p) t -> p k t", p=128)
        xq = (nc.sync, nc.scalar)
        for tci in range(8):
            xq[tci % 2].dma_start(
                out=xt_sb[:, :, 256 * tci : 256 * (tci + 1)],
                in_=xT_r[:, :, 256 * tci : 256 * (tci + 1)],
            )
        nc.gpsimd.dma_start(
            out=wo, in_=wo_d.rearrange("(c p) o -> p c o", p=128)
        )

        # denominator columns for the PV matmul; 32.0 cancels the 32x
        # weight pre-scale carried by V
        for h in range(HPC):
            nc.vector.memset(v_sb[:, :, 66 * h + 64 : 66 * h + 65], 32.0)

        expp = top.enter_context(tc.tile_pool(name="expp", bufs=4))
        smallp = top.enter_context(tc.tile_pool(name="smallp", bufs=2))
        outp = top.enter_context(tc.tile_pool(name="outp", bufs=3))
        # 8 PSUM banks: S pair tiles 2x2, ctx accumulators 2x1, filler 2x1.
        # The deferred normalize frees ctx slots within ~1 iteration, so the
        # ctx ring gets away with 2; the filler ring needs 2 so a chunk's
        # matmuls overlap the previous chunk's PSUM->SBUF copy.
        psS = top.enter_context(tc.tile_pool(name="psS", bufs=2, space="PSUM"))
        psC = top.enter_context(tc.tile_pool(name="psC", bufs=2, space="PSUM"))
        psF = top.enter_context(tc.tile_pool(name="psF", bufs=2, space="PSUM"))

        # ---------------- filler chunk builders ----------------
        # Each builder returns (matmul_thunks, finish_thunk); the filler
        # queue steps one matmul at a time between the S and PV matmuls of
        # the attention loop so the PE never idles on the exp stream.
        state = {"v_done": -1, "qk0_cnt": 0, "mul3_done": -1}

        def v_chunk(t):
            def build():
                ps = psF.tile([128, 512], f32, tag="fill", name=f"psv_{t}")
                mms = [
                    (lambda k=k: nc.tensor.matmul(
                        ps,
                        xt_sb[:, 2 * k : 2 * k + 2, 128 * t : 128 * (t + 1)],
                        wv[:, 2 * k : 2 * k + 2, :],
                        start=(k == 0),
                        stop=(k == KT // 2 - 1),
                        perf_mode=DR,
                    ))
                    for k in range(KT // 2)
                ]

                def fin():
                    nc.vector.tensor_copy(
                        v_sb[:, t, :].rearrange("p (h c) -> p h c", c=HD + 2)[
                            :, :, 0:HD
                        ],
                        ps.rearrange("p (h c) -> p h c", c=HD),
                    )
                    state["v_done"] = t

                return mms, fin

            return build

        def qk_chunk(m, w_sb, dest, ci, count=False):
            def build():
                ps = psF.tile([128, 512], f32, tag="fill", name=f"psqk_{m}_{ci}")
                mms = [
                    (lambda k=k: nc.tensor.matmul(
                        ps,
                        w_sb[:, 2 * k : 2 * k + 2, 128 * m : 128 * (m + 1)],
                        xt_sb[:, 2 * k : 2 * k + 2, 512 * ci : 512 * (ci + 1)],
                        start=(k == 0),
                        stop=(k == KT // 2 - 1),
                        perf_mode=DR,
                    ))
                    for k in range(KT // 2)
                ]

                def fin():
                    nc.vector.tensor_copy(
                        dest[:, m, 512 * ci : 512 * (ci + 1)], ps
                    )
                    if count:
                        state["qk0_cnt"] += 1

                return mms, fin

            return build

        def out_chunk(tt, oc):
            def build():
                ps = psF.tile([128, 512], f32, tag="fill", name=f"pso_{tt}_{oc}")
                mms = [
                    (lambda ct=ct: nc.tensor.matmul(
                        ps,
                        ctx_sb[:, ct, 128 * tt : 128 * (tt + 1)],
                        wo[:, ct, 512 * oc : 512 * (oc + 1)],
                        start=(ct == 0),
                        stop=(ct == 3),
                    ))
                    for ct in range(4)
                ]

                def fin():
                    ot = outp.tile([128, 512], f32, tag="ot", name=f"ot_{tt}_{oc}")
                    nc.vector.tensor_copy(ot, ps)
                    nc.sync.dma_start(
                        out=out_d[
                            128 * tt : 128 * (tt + 1), 512 * oc : 512 * (oc + 1)
                        ],
                        in_=ot,
                    )

                return mms, fin

            return build

        class Filler:
            """Queue of (gate, chunk-builder); step() emits one matmul."""

            def __init__(self, chunks):
                self.chunks = chunks
                self.ci = 0
                self.mi = 0
                self.cur = None
                self.emitted = 0
                self.total = sum(n for _, n, _ in chunks)

            def step(self, idx):
                if self.ci >= len(self.chunks):
                    return False
                gate, _, build = self.chunks[self.ci]
                if callable(gate):
                    if not gate():
                        return False
                elif idx < gate:
                    return False
                if self.cur is None:
                    self.cur = build()
                mms, fin = self.cur
                mms[self.mi]()
                self.mi += 1
                self.emitted += 1
                if self.mi == len(mms):
                    fin()
                    self.cur = None
                    self.mi = 0
                    self.ci += 1
                return True

            def pace(self, idx, n_iters):
                target = (self.total * (idx + 1) + n_iters - 1) // n_iters
                while self.emitted < target and self.step(idx):
                    pass

            def drain(self):
                while self.step(1 << 30):
                    pass

        # ---------------- normalize ----------------
        # The raw copy frees the ctx PSUM slot immediately; the remaining
        # stages (approx-reciprocal of the denominator row, gpsimd partition
        # broadcast, divide-multiply into ctx_sb) are pushed onto a lazy
        # queue and popped a couple of iterations later, when their inputs
        # have long completed -- an eagerly-emitted chain would head-of-line
        # block the DVE/gpsimd FIFOs behind its cross-engine waits.
        from collections import deque

        normq = deque()

        def normalize(ctx_ps, h, c):
            dq = h // 2
            pr = (h % 2) * 64
            raw = smallp.tile([65, 512], f32, tag="raw", bufs=4,
                              name=f"raw_{h}_{c}")
            nc.vector.tensor_copy(raw, ctx_ps)
            sp = smallp.tile([128, 4], f32, tag="sp", bufs=3,
                             name=f"sp_{h}_{c}")
            rp = smallp.tile([128, 4], f32, tag="rp", bufs=3,
                             name=f"rp_{h}_{c}")
            recip = smallp.tile([1, 512], f32, tag="recip", bufs=3,
                                name=f"rc_{h}_{c}")
            bc = smallp.tile([64, 512], f32, tag="bc", bufs=3,
                             name=f"bc_{h}_{c}")
            # the [1, 512] denominator row would cost ~8 cyc/elem on a single
            # DVE lane; spread it over 128 lanes with two tiny DMA reshapes
            normq.append(lambda: nc.sync.dma_start(out=sp, in_=raw[64:65, :]))
            normq.append(lambda: nc.vector.reciprocal(rp, sp))
            normq.append(lambda: nc.sync.dma_start(out=recip, in_=rp))

            normq.append(lambda: nc.gpsimd.partition_broadcast(bc, recip))

            def mul_stage():
                nc.vector.tensor_mul(
                    ctx_sb[pr : pr + 64, dq, 512 * c : 512 * (c + 1)],
                    raw[0:64, :],
                    bc,
                )
                if dq == 3 and pr == 64:
                    # both halves of pair-3 chunk c are now written; unblocks
                    # the out-projection filler chunks that read them
                    state["mul3_done"] = c

            normq.append(mul_stage)

        # ---------------- attention for one head pair ----------------
        def attention(m, filler):
            iters = [(c, t) for c in range(4) for t in range(4 * c + 4)]

            def emit_S(c, t):
                if m == 0:
                    # pair-0 Q/K projection chunks stream through the filler;
                    # chunk c of the t-loop needs projection chunks <= c
                    while 1 + state["qk0_cnt"] // 2 <= c and filler.step(1 << 30):
                        pass
                    assert 1 + state["qk0_cnt"] // 2 > c
                i0 = max(128 * t, 512 * c)
                ext = 512 * (c + 1) - i0
                sp = psS.tile([128, 1024], f32, tag="sps",
                              name=f"sps_{m}_{c}_{t}")
                for half in range(2):
                    pr = 64 * half
                    nc.tensor.matmul(
                        sp[:, 512 * half : 512 * half + ext],
                        kt[pr : pr + 64, m, 128 * t : 128 * (t + 1)],
                        qt[pr : pr + 64, m, i0 : i0 + ext],
                        start=True,
                        stop=True,
                    )
                return sp

            ctx_pair = [None, None]
            es4 = None
            base = 0
            sp_next = emit_S(*iters[0])
            for idx, (c, t) in enumerate(iters):
                sp = sp_next
                i0 = max(128 * t, 512 * c)
                ext = 512 * (c + 1) - i0
                if t % 2 == 0:
                    # one es tile per t-pair: [p, t-slot, head-half, 512];
                    # the PV matmul contracts both t-slots in one DoubleRow
                    # pass
                    es = expp.tile([128, 2048], f8, tag="es",
                                   name=f"es_{m}_{c}_{t}")
                    es4 = es.rearrange("p (u g x) -> p u g x", u=2, g=2)
                    base = i0
                    if t >= 4 * c:
                        # diagonal-region pair: the odd slot's leading 128
                        # columns are never written by its exp; zero them so
                        # the PV contraction ignores them
                        nc.vector.memset(es4[:, 1, :, 0:128], 0.0)
                off = i0 - base
                nc.scalar.activation(
                    es4[:, t % 2, :, off : off + ext],
                    sp.rearrange("p (g x) -> p g x", g=2)[:, :, 0:ext],
                    Exp,
                    scale=0.125 / 1024.0,
                )
                if i0 == 128 * t:
                    # keep element iff free_idx - partition_idx >= 0, applied
                    # to the diagonal 128-block of both heads in one call
                    nc.gpsimd.affine_select(
                        out=es4[:, t % 2, :, off : off + 128],
                        in_=es4[:, t % 2, :, off : off + 128],
                        compare_op=mybir.AluOpType.is_ge,
                        fill=0.0,
                        base=0,
                        pattern=[[0, 2], [1, 128]],
                        channel_multiplier=-1,
                    )
                # S one iteration ahead of PV, so the exp stream always has
                # material while the PE works through filler matmuls
                if idx + 1 < len(iters):
                    sp_next = emit_S(*iters[idx + 1])
                if m == 0:
                    while state["v_done"] < t and filler.step(idx):
                        pass
                    assert state["v_done"] >= t
                filler.pace(idx, len(iters))
                if normq:
                    normq.popleft()()
                if t % 2 == 1:
                    u = t // 2
                    if u == 0:
                        for half in range(2):
                            ctx_pair[half] = psC.tile(
                                [65, 512], f32, tag="ctxps",
                                name=f"ctxps_{m}_{c}_{half}",
                            )
                    ext0 = 512 * (c + 1) - base
                    for half in range(2):
                        h = 2 * m + half
                        nc.tensor.matmul(
                            ctx_pair[half][:, base - 512 * c : 512 * (c + 1) - 512 * c],
                            v_sb[:, t - 1 : t + 1, 66 * h : 66 * h + 65],
                            es4[:, :, half, 0:ext0],
                            start=(u == 0),
                            stop=(u == 2 * c + 1),
                            perf_mode=DR,
                        )
                if t == 4 * c + 3:
                    for half in range(2):
                        normalize(ctx_pair[half], 2 * m + half, c)

        # ---------------- emission schedule ----------------
        # upfront: V tiles 0..3 and the first pair-0 Q/K projection chunk;
        # everything else drip-feeds through the attention loops
        f0 = Filler(
            [(0, KT // 2, v_chunk(t)) for t in range(4)]
            + [(0, KT // 2, qk_chunk(0, w, d, 0)) for w, d in ((wq, qt), (wk, kt))]
        )
        f0.drain()

        # iteration index at which query-chunk c of the t-loop is finished
        cend = [4, 12, 24, 40]
        fillers = [
            Filler(
                # ordered by consumption deadline in the pair-0 t-loop
                [(0, KT // 2, qk_chunk(0, w, d, 1, count=True))
                 for w, d in ((wq, qt), (wk, kt))]
                + [(0, KT // 2, v_chunk(t)) for t in range(4, 8)]
                + [(0, KT // 2, qk_chunk(0, w, d, 2, count=True))
                   for w, d in ((wq, qt), (wk, kt))]
                + [(0, KT // 2, v_chunk(t)) for t in range(8, 12)]
                + [(0, KT // 2, qk_chunk(0, w, d, 3, count=True))
                   for w, d in ((wq, qt), (wk, kt))]
                + [(0, KT // 2, v_chunk(t)) for t in range(12, NT)]
                + [(0, KT // 2, qk_chunk(1, w, d, ci))
                   for w, d in ((wq, qt), (wk, kt)) for ci in range(4)]
            ),
            Filler([(0, KT // 2, qk_chunk(2, w, d, ci))
                    for w, d in ((wq, qt), (wk, kt)) for ci in range(4)]
                   + [(0, KT // 2, qk_chunk(3, w, d, ci))
                      for w, d in ((wq, qt), (wk, kt)) for ci in range(2)]),
            Filler([(0, KT // 2, qk_chunk(3, w, d, ci))
                    for w, d in ((wq, qt), (wk, kt)) for ci in range(2, 4)]),
            # out-proj of token tile tt needs the pair-3 lazy normalize of
            # query chunk tt//4 to have drained from the queue
            Filler([((lambda cc=tt // 4: state["mul3_done"] >= cc), 4,
                     out_chunk(tt, oc))
                    for tt in range(12) for oc in range(2)]),
        ]
        for m in range(3):
            attention(m, fillers[m])
            fillers[m].drain()
        attention(3, fillers[3])
        # flush the remaining (pair-3 c=3) normalize stages before the PE
        # drain work so the reciprocal chain overlaps it
        while normq:
            normq.popleft()()
        fillers[3].drain()
        # tail: remaining out-proj tiles, double-wide on the freed psS ring
        # so consecutive chunks pipeline instead of serializing on the
        # single-buffer filler PSUM
        for tt in range(12, NT):
            ps = psS.tile([128, 1024], f32, tag="sps", name=f"psob_{tt}")
            for oc in range(2):
                for ct in range(4):
                    nc.tensor.matmul(
                        ps[:, 512 * oc : 512 * (oc + 1)],
                        ctx_sb[:, ct, 128 * tt : 128 * (tt + 1)],
                        wo[:, ct, 512 * oc : 512 * (oc + 1)],
                        start=(ct == 0),
                        stop=(ct == 3),
                    )
            ot = outp.tile([128, 1024], f32, tag="otb", name=f"otb_{tt}")
            nc.vector.tensor_copy(ot, ps)
            nc.sync.dma_start(out=out_d[128 * tt : 128 * (tt + 1), :], in_=ot)

    nc.compile()
    return nc


def _get_program():
    if "nc" not in _CACHE:
        _CACHE["nc"] = _build_program()
    return _CACHE["nc"]


def make_in_maps(x, Wq, Wk, Wv, Wo):
    import ml_dtypes

    bf16 = ml_dtypes.bfloat16
    f8 = ml_dtypes.float8_e4m3
    in_maps = []
    for core in range(NCORES):
        b, hg = core // 2, core % 2
        sl = slice(DPC * hg, DPC * (hg + 1))
        in_maps.append(
            {
                "xT": np.ascontiguousarray(x[b].T).astype(f8),
                # 32x pre-scale lifts the ~N(0, 0.02) weights out of the
                # fp8e4m3 subnormal range; undone on-device (see kernel doc)
                "wq": np.ascontiguousarray(32.0 * Wq[:, sl]).astype(f8),
                "wk": np.ascontiguousarray(32.0 * Wk[:, sl]).astype(f8),
                "wv": np.ascontiguousarray(32.0 * Wv[:, sl]).astype(f8),
                "wo": np.ascontiguousarray(Wo[sl, :]).astype(bf16),
            }
        )
    return in_maps


def kernel(x, Wq, Wk, Wv, Wo, bo):
    global LAST_RESULTS
    from concourse.bass_utils import run_bass_kernel_spmd

    x = np.asarray(x, dtype=np.float32)
    nc = _get_program()
    in_maps = make_in_maps(
        x,
        np.asarray(Wq, np.float32),
        np.asarray(Wk, np.float32),
        np.asarray(Wv, np.float32),
        np.asarray(Wo, np.float32),
    )
    res = run_bass_kernel_spmd(
        nc,
        in_maps,
        list(range(NCORES)),
        trace=bool(int(os.environ.get("KERNEL_TRACE", "0"))),
    )
    LAST_RESULTS = res
    bo = np.asarray(bo, np.float32)
    out = np.empty((B, T, D), np.float32)
    for b in range(B):
        out[b] = res.results[2 * b]["out"] + res.results[2 * b + 1]["out"] + bo
    return out


# revision 49
# speedup vs baseline: 1.0194x; 1.0194x over previous
"""Multi-head causal attention (B=4, T=2048, D=1024, H=16, hd=64) on 8 trn2 cores.

Sharding: core = (batch, head_group): 4 batches x 2 head-groups of 8 heads.
Each core computes its batch's attention for its 8 heads plus the partial
output projection; the host sums the two head-group partials per batch and
adds the output bias.

Per-core kernel (fp32 PSUM accumulation throughout):
  x and Wq/Wk/Wv live in fp8e4m3 (weights pre-scaled 32x on the host to
  escape the e4m3 subnormal range); Q/K/V projections run fp8 DoubleRow
  matmuls (two k-tiles contracted per pass).  The 32*32 score scale is
  undone inside the softmax exp scale; the 32x on V cancels against a 32.0
  denominator-ones column.
  per head-pair m, query chunk c (512 wide), j-tile t:
    S^T pair: two row-tiled bf16 matmuls (contract rows 0-63 / 64-127)
              issued back-to-back -> run concurrently on the PE, into one
              [128, 2x512] PSUM tile
    expS = exp(S^T / (8*1024))  one ACT instruction for both heads, ->fp8
    causal mask on the diagonal block (one GPSIMD affine_select, both heads)
    per t-PAIR: ctxT_aug += V_aug^T-contract expS, a single fp8 DoubleRow
              matmul contracting both j-tiles (row 64 = denominator)
  ctx = ctxT[0:64] * (1/denom)          (lazy DVE/GPSIMD chain, see below)
  out_partial = ctx^T-contract Wo_s     [2048, 1024] in bf16

The scalar-engine exp stream is the attention-phase bottleneck, so the
tensor engine is kept busy through its exp waits: the S matmuls run one
iteration ahead of the PV matmuls, and projection work (V tiles, the next
pair's Q/K projection, the output projection) is drip-fed between S and PV
one matmul at a time from a chunk-filler queue.  The normalize chains are
emitted lazily, one stage per iteration, so their cross-engine waits never
head-of-line block an engine FIFO.
"""

import os
import sys

sys.path.insert(0, "/opt/trn_rl_repo")

import numpy as np

B = 4
T = 2048
D = 1024
H = 16
HD = 64
NCORES = 8
HPC = 8          # heads per core
DPC = HPC * HD   # 512
KT = D // 128    # 8 k-tiles
NT = T // 128    # 16 token tiles

_CACHE = {}
LAST_RESULTS = None


def _build_program():
    from contextlib import ExitStack

    import concourse.bass as bass
    import concourse.tile as tile
    from concourse import bacc, mybir

    f32 = mybir.dt.float32
    bf16 = mybir.dt.bfloat16
    f8 = mybir.dt.float8e4
    DR = mybir.MatmulPerfMode.DoubleRow
    Exp = mybir.ActivationFunctionType.Exp

    nc = bacc.Bacc(
        "TRN2", target_bir_lowering=False, debug=False, num_devices=NCORES
    )
    # x and the projection weights are fp8 (weights pre-scaled by 32 on the
    # host so their ~N(0, 0.02) values escape the e4m3 subnormal range); the
    # 32*32 scale on the scores is undone in the softmax exp scale, and the
    # 32x on V cancels against a 32.0 denominator-ones column
    xT = nc.dram_tensor("xT", [D, T], f8, kind="ExternalInput").ap()
    wq_d = nc.dram_tensor("wq", [D, DPC], f8, kind="ExternalInput").ap()
    wk_d = nc.dram_tensor("wk", [D, DPC], f8, kind="ExternalInput").ap()
    wv_d = nc.dram_tensor("wv", [D, DPC], f8, kind="ExternalInput").ap()
    wo_d = nc.dram_tensor("wo", [DPC, D], f8, kind="ExternalInput").ap()
    out_d = nc.dram_tensor("out", [T, D], f32, kind="ExternalOutput").ap()

    with tile.TileContext(nc) as tc, ExitStack() as top:
        persist = top.enter_context(tc.tile_pool(name="persist", bufs=1))
        xt_sb = persist.tile([128, KT, T], f8, tag="xt")
        qt = persist.tile([128, 4, T], bf16, tag="qt")
        kt = persist.tile([128, 4, T], bf16, tag="kt")
        # per-tile row stride 8*66=528 bytes keeps the t-pair step 16-aligned
        # for the DoubleRow PV weight AP; head h occupies cols 66h..66h+64
        v_sb = persist.tile([128, NT, HPC * (HD + 2)], f8, tag="v")
        ctx_sb = persist.tile([128, 4, T], f8, tag="ctx")
        wq = persist.tile([128, KT, DPC], f8, tag="wq")
        wk = persist.tile([128, KT, DPC], f8, tag="wk")
        wv = persist.tile([128, KT, DPC], f8, tag="wv")
        wo = persist.tile([128, 4, D], f8, tag="wo")

        # weight loads on the gpsimd queue so they overlap the x-chunk loads
        # on the sync/scalar queues; wv is split per k-pair so the first V
        # matmul starts as soon as the first quarter lands
        for kj in range(4):
            nc.gpsimd.dma_start(
                out=wv[:, 2 * kj : 2 * kj + 2, :],
                in_=wv_d.rearrange("(k p) d -> p k d", p=128)[
                    :, 2 * kj : 2 * kj + 2, :
                ],
            )
        for w_sb, w_d in ((wq, wq_d), (wk, wk_d)):
            nc.gpsimd.dma_start(
                out=w_sb, in_=w_d.rearrange("(k p) d -> p k d", p=128)
            )
        xT_r = xT.rearrange("(k p) t -> p k t", p=128)
        xq = (nc.sync, nc.scalar)
        for tci in range(8):
            xq[tci % 2].dma_start(
                out=xt_sb[:, :, 256 * tci : 256 * (tci + 1)],
                in_=xT_r[:, :, 256 * tci : 256 * (tci + 1)],
            )
        nc.gpsimd.dma_start(
            out=wo, in_=wo_d.rearrange("(c p) o -> p c o", p=128)
        )

        # denominator columns for the PV matmul; 32.0 cancels the 32x
        # weight pre-scale carried by V
        for h in range(HPC):
            nc.vector.memset(v_sb[:, :, 66 * h + 64 : 66 * h + 65], 32.0)

        expp = top.enter_context(tc.tile_pool(name="expp", bufs=4))
        smallp = top.enter_context(tc.tile_pool(name="smallp", bufs=2))
        outp = top.enter_context(tc.tile_pool(name="outp", bufs=3))
        # 8 PSUM banks: S pair tiles 2x2, ctx accumulators 2x1, filler 2x1.
        # The deferred normalize frees ctx slots within ~1 iteration, so the
        # ctx ring gets away with 2; the filler ring needs 2 so a chunk's
        # matmuls overlap the previous chunk's PSUM->SBUF copy.
        psS = top.enter_context(tc.tile_pool(name="psS", bufs=2, space="PSUM"))
        psC = top.enter_context(tc.tile_pool(name="psC", bufs=2, space="PSUM"))
        psF = top.enter_context(tc.tile_pool(name="psF", bufs=2, space="PSUM"))

        # ---------------- filler chunk builders ----------------
        # Each builder returns (matmul_thunks, finish_thunk); the filler
        # queue steps one matmul at a time between the S and PV matmuls of
        # the attention loop so the PE never idles on the exp stream.
        state = {"v_done": -1, "qk0_cnt": 0, "mul3_done": -1}

        def v_chunk(t):
            def build():
                ps = psF.tile([128, 512], f32, tag="fill", name=f"psv_{t}")
                mms = [
                    (lambda k=k: nc.tensor.matmul(
                        ps,
                        xt_sb[:, 2 * k : 2 * k + 2, 128 * t : 128 * (t + 1)],
                        wv[:, 2 * k : 2 * k + 2, :],
                        start=(k == 0),
                        stop=(k == KT // 2 - 1),
                        perf_mode=DR,
                    ))
                    for k in range(KT // 2)
                ]

                def fin():
                    nc.vector.tensor_copy(
                        v_sb[:, t, :].rearrange("p (h c) -> p h c", c=HD + 2)[
                            :, :, 0:HD
                        ],
                        ps.rearrange("p (h c) -> p h c", c=HD),
                    )
                    state["v_done"] = t

                return mms, fin

            return build

        def qk_chunk(m, w_sb, dest, ci, count=False):
            def build():
                ps = psF.tile([128, 512], f32, tag="fill", name=f"psqk_{m}_{ci}")
                mms = [
                    (lambda k=k: nc.tensor.matmul(
                        ps,
                        w_sb[:, 2 * k : 2 * k + 2, 128 * m : 128 * (m + 1)],
                        xt_sb[:, 2 * k : 2 * k + 2, 512 * ci : 512 * (ci + 1)],
                        start=(k == 0),
                        stop=(k == KT // 2 - 1),
                        perf_mode=DR,
                    ))
                    for k in range(KT // 2)
                ]

                def fin():
                    nc.vector.tensor_copy(
                        dest[:, m, 512 * ci : 512 * (ci + 1)], ps
                    )
                    if count:
                        state["qk0_cnt"] += 1

                return mms, fin

            return build

        def out_chunk(tt, oc):
            def build():
                ps = psF.tile([128, 512], f32, tag="fill", name=f"pso_{tt}_{oc}")
                mms = [
                    (lambda j=j: nc.tensor.matmul(
                        ps,
                        ctx_sb[:, 2 * j : 2 * j + 2, 128 * tt : 128 * (tt + 1)],
                        wo[:, 2 * j : 2 * j + 2, 512 * oc : 512 * (oc + 1)],
                        start=(j == 0),
                        stop=(j == 1),
                        perf_mode=DR,
                    ))
                    for j in range(2)
                ]

                def fin():
                    ot = outp.tile([128, 512], f32, tag="ot", name=f"ot_{tt}_{oc}")
                    nc.vector.tensor_scalar_mul(ot, ps, 1.0 / 32.0)
                    nc.sync.dma_start(
                        out=out_d[
                            128 * tt : 128 * (tt + 1), 512 * oc : 512 * (oc + 1)
                        ],
                        in_=ot,
                    )

                return mms, fin

            return build

        class Filler:
            """Queue of (gate, chunk-builder); step() emits one matmul."""

            def __init__(self, chunks):
                self.chunks = chunks
                self.ci = 0
                self.mi = 0
                self.cur = None
                self.emitted = 0
                self.total = sum(n for _, n, _ in chunks)

            def step(self, idx):
                if self.ci >= len(self.chunks):
                    return False
                gate, _, build = self.chunks[self.ci]
                if callable(gate):
                    if not gate():
                        return False
                elif idx < gate:
                    return False
                if self.cur is None:
                    self.cur = build()
                mms, fin = self.cur
                mms[self.mi]()
                self.mi += 1
                self.emitted += 1
                if self.mi == len(mms):
                    fin()
                    self.cur = None
                    self.mi = 0
                    self.ci += 1
                return True

            def pace(self, idx, n_iters):
                target = (self.total * (idx + 1) + n_iters - 1) // n_iters
                while self.emitted < target and self.step(idx):
                    pass

            def drain(self):
                while self.step(1 << 30):
                    pass

        # ---------------- normalize ----------------
        # The raw copy frees the ctx PSUM slot immediately; the remaining
        # stages (approx-reciprocal of the denominator row, gpsimd partition
        # broadcast, divide-multiply into ctx_sb) are pushed onto a lazy
        # queue and popped a couple of iterations later, when their inputs
        # have long completed -- an eagerly-emitted chain would head-of-line
        # block the DVE/gpsimd FIFOs behind its cross-engine waits.
        from collections import deque

        normq = deque()

        def normalize(ctx_ps, h, c):
            dq = h // 2
            pr = (h % 2) * 64
            raw = smallp.tile([65, 512], f32, tag="raw", bufs=4,
                              name=f"raw_{h}_{c}")
            nc.vector.tensor_copy(raw, ctx_ps)
            sp = smallp.tile([128, 4], f32, tag="sp", bufs=3,
                             name=f"sp_{h}_{c}")
            rp = smallp.tile([128, 4], f32, tag="rp", bufs=3,
                             name=f"rp_{h}_{c}")
            recip = smallp.tile([1, 512], f32, tag="recip", bufs=3,
                                name=f"rc_{h}_{c}")
            bc = smallp.tile([64, 512], f32, tag="bc", bufs=3,
                             name=f"bc_{h}_{c}")
            # the [1, 512] denominator row would cost ~8 cyc/elem on a single
            # DVE lane; spread it over 128 lanes with two tiny DMA reshapes
            normq.append(lambda: nc.sync.dma_start(out=sp, in_=raw[64:65, :]))
            normq.append(lambda: nc.vector.reciprocal(rp, sp))
            normq.append(lambda: nc.sync.dma_start(out=recip, in_=rp))

            normq.append(lambda: nc.gpsimd.partition_broadcast(bc, recip))

            def mul_stage():
                nc.vector.tensor_mul(
                    ctx_sb[pr : pr + 64, dq, 512 * c : 512 * (c + 1)],
                    raw[0:64, :],
                    bc,
                )
                if dq == 3 and pr == 64:
                    # both halves of pair-3 chunk c are now written; unblocks
                    # the out-projection filler chunks that read them
                    state["mul3_done"] = c

            normq.append(mul_stage)

        # ---------------- attention for one head pair ----------------
        def attention(m, filler):
            iters = [(c, t) for c in range(4) for t in range(4 * c + 4)]

            def emit_S(c, t):
                if m == 0:
                    # pair-0 Q/K projection chunks stream through the filler;
                    # chunk c of the t-loop needs projection chunks <= c
                    while 1 + state["qk0_cnt"] // 2 <= c and filler.step(1 << 30):
                        pass
                    assert 1 + state["qk0_cnt"] // 2 > c
                i0 = max(128 * t, 512 * c)
                ext = 512 * (c + 1) - i0
                sp = psS.tile([128, 1024], f32, tag="sps",
                              name=f"sps_{m}_{c}_{t}")
                for half in range(2):
                    pr = 64 * half
                    nc.tensor.matmul(
                        sp[:, 512 * half : 512 * half + ext],
                        kt[pr : pr + 64, m, 128 * t : 128 * (t + 1)],
                        qt[pr : pr + 64, m, i0 : i0 + ext],
                        start=True,
                        stop=True,
                    )
                return sp

            ctx_pair = [None, None]
            es4 = None
            base = 0
            sp_next = emit_S(*iters[0])
            for idx, (c, t) in enumerate(iters):
                sp = sp_next
                i0 = max(128 * t, 512 * c)
                ext = 512 * (c + 1) - i0
                if t % 2 == 0:
                    # one es tile per t-pair: [p, t-slot, head-half, 512];
                    # the PV matmul contracts both t-slots in one DoubleRow
                    # pass
                    es = expp.tile([128, 2048], f8, tag="es",
                                   name=f"es_{m}_{c}_{t}")
                    es4 = es.rearrange("p (u g x) -> p u g x", u=2, g=2)
                    base = i0
                    if t >= 4 * c:
                        # diagonal-region pair: the odd slot's leading 128
                        # columns are never written by its exp; zero them so
                        # the PV contraction ignores them
                        nc.vector.memset(es4[:, 1, :, 0:128], 0.0)
                off = i0 - base
                nc.scalar.activation(
                    es4[:, t % 2, :, off : off + ext],
                    sp.rearrange("p (g x) -> p g x", g=2)[:, :, 0:ext],
                    Exp,
                    scale=0.125 / 1024.0,
                )
                if i0 == 128 * t:
                    # keep element iff free_idx - partition_idx >= 0, applied
                    # to the diagonal 128-block of both heads in one call
                    nc.gpsimd.affine_select(
                        out=es4[:, t % 2, :, off : off + 128],
                        in_=es4[:, t % 2, :, off : off + 128],
                        compare_op=mybir.AluOpType.is_ge,
                        fill=0.0,
                        base=0,
                        pattern=[[0, 2], [1, 128]],
                        channel_multiplier=-1,
                    )
                # S one iteration ahead of PV, so the exp stream always has
                # material while the PE works through filler matmuls
                if idx + 1 < len(iters):
                    sp_next = emit_S(*iters[idx + 1])
                if m == 0:
                    while state["v_done"] < t and filler.step(idx):
                        pass
                    assert state["v_done"] >= t
                filler.pace(idx, len(iters))
                if normq:
                    normq.popleft()()
                if t % 2 == 1:
                    u = t // 2
                    if u == 0:
                        for half in range(2):
                            ctx_pair[half] = psC.tile(
                                [65, 512], f32, tag="ctxps",
                                name=f"ctxps_{m}_{c}_{half}",
                            )
                    ext0 = 512 * (c + 1) - base
                    for half in range(2):
                        h = 2 * m + half
                        nc.tensor.matmul(
                            ctx_pair[half][:, base - 512 * c : 512 * (c + 1) - 512 * c],
                            v_sb[:, t - 1 : t + 1, 66 * h : 66 * h + 65],
                            es4[:, :, half, 0:ext0],
                            start=(u == 0),
                            stop=(u == 2 * c + 1),
                            perf_mode=DR,
                        )
                if t == 4 * c + 3:
                    for half in range(2):
                        normalize(ctx_pair[half], 2 * m + half, c)

        # ---------------- emission schedule ----------------
        # upfront: V tiles 0..3 and the first pair-0 Q/K projection chunk;
        # everything else drip-feeds through the attention loops
        f0 = Filler(
            [(0, KT // 2, v_chunk(t)) for t in range(4)]
            + [(0, KT // 2, qk_chunk(0, w, d, 0)) for w, d in ((wq, qt), (wk, kt))]
        )
        f0.drain()

        # iteration index at which query-chunk c of the t-loop is finished
        cend = [4, 12, 24, 40]
        fillers = [
            Filler(
                # ordered by consumption deadline in the pair-0 t-loop
                [(0, KT // 2, qk_chunk(0, w, d, 1, count=True))
                 for w, d in ((wq, qt), (wk, kt))]
                + [(0, KT // 2, v_chunk(t)) for t in range(4, 8)]
                + [(0, KT // 2, qk_chunk(0, w, d, 2, count=True))
                   for w, d in ((wq, qt), (wk, kt))]
                + [(0, KT // 2, v_chunk(t)) for t in range(8, 12)]
                + [(0, KT // 2, qk_chunk(0, w, d, 3, count=True))
                   for w, d in ((wq, qt), (wk, kt))]
                + [(0, KT // 2, v_chunk(t)) for t in range(12, NT)]
                + [(0, KT // 2, qk_chunk(1, w, d, ci))
                   for w, d in ((wq, qt), (wk, kt)) for ci in range(4)]
            ),
            Filler([(0, KT // 2, qk_chunk(2, w, d, ci))
                    for w, d in ((wq, qt), (wk, kt)) for ci in range(4)]
                   + [(0, KT // 2, qk_chunk(3, w, d, ci))
                      for w, d in ((wq, qt), (wk, kt)) for ci in range(2)]),
            Filler([(0, KT // 2, qk_chunk(3, w, d, ci))
                    for w, d in ((wq, qt), (wk, kt)) for ci in range(2, 4)]),
            # out-proj of token tile tt needs the pair-3 lazy normalize of
            # query chunk tt//4 to have drained from the queue
            Filler([((lambda cc=tt // 4: state["mul3_done"] >= cc), 2,
                     out_chunk(tt, oc))
                    for tt in range(12) for oc in range(2)]),
        ]
        for m in range(3):
            attention(m, fillers[m])
            fillers[m].drain()
        attention(3, fillers[3])
        # flush the remaining (pair-3 c=3) normalize stages before the PE
        # drain work so the reciprocal chain overlaps it
        while normq:
            normq.popleft()()
        fillers[3].drain()
        # tail: remaining out-proj tiles, double-wide on the freed psS ring
        # so consecutive chunks pipeline instead of serializing on the
        # single-buffer filler PSUM
        for tt in range(12, NT):
            ps = psS.tile([128, 1024], f32, tag="sps", name=f"psob_{tt}")
            for oc in range(2):
                for j in range(2):
                    nc.tensor.matmul(
                        ps[:, 512 * oc : 512 * (oc + 1)],
                        ctx_sb[:, 2 * j : 2 * j + 2, 128 * tt : 128 * (tt + 1)],
                        wo[:, 2 * j : 2 * j + 2, 512 * oc : 512 * (oc + 1)],
                        start=(j == 0),
                        stop=(j == 1),
                        perf_mode=DR,
                    )
            ot = outp.tile([128, 1024], f32, tag="otb", name=f"otb_{tt}")
            nc.vector.tensor_scalar_mul(ot, ps, 1.0 / 32.0)
            nc.sync.dma_start(out=out_d[128 * tt : 128 * (tt + 1), :], in_=ot)

    nc.compile()
    return nc


def _get_program():
    if "nc" not in _CACHE:
        _CACHE["nc"] = _build_program()
    return _CACHE["nc"]


def make_in_maps(x, Wq, Wk, Wv, Wo):
    import ml_dtypes

    bf16 = ml_dtypes.bfloat16
    f8 = ml_dtypes.float8_e4m3
    in_maps = []
    for core in range(NCORES):
        b, hg = core // 2, core % 2
        sl = slice(DPC * hg, DPC * (hg + 1))
        in_maps.append(
            {
                "xT": np.ascontiguousarray(x[b].T).astype(f8),
                # 32x pre-scale lifts the ~N(0, 0.02) weights out of the
                # fp8e4m3 subnormal range; undone on-device (see kernel doc)
                "wq": np.ascontiguousarray(32.0 * Wq[:, sl]).astype(f8),
                "wk": np.ascontiguousarray(32.0 * Wk[:, sl]).astype(f8),
                "wv": np.ascontiguousarray(32.0 * Wv[:, sl]).astype(f8),
                "wo": np.ascontiguousarray(32.0 * Wo[sl, :]).astype(f8),
            }
        )
    return in_maps


def kernel(x, Wq, Wk, Wv, Wo, bo):
    global LAST_RESULTS
    from concourse.bass_utils import run_bass_kernel_spmd

    x = np.asarray(x, dtype=np.float32)
    nc = _get_program()
    in_maps = make_in_maps(
        x,
        np.asarray(Wq, np.float32),
        np.asarray(Wk, np.float32),
        np.asarray(Wv, np.float32),
        np.asarray(Wo, np.float32),
    )
    res = run_bass_kernel_spmd(
        nc,
        in_maps,
        list(range(NCORES)),
        trace=bool(int(os.environ.get("KERNEL_TRACE", "0"))),
    )
    LAST_RESULTS = res
    bo = np.asarray(bo, np.float32)
    out = np.empty((B, T, D), np.float32)
    for b in range(B):
        out[b] = res.results[2 * b]["out"] + res.results[2 * b + 1]["out"] + bo
    return out


# revision 51
# speedup vs baseline: 1.2066x; 1.1836x over previous
"""Multi-head causal attention (B=4, T=2048, D=1024, H=16, hd=64) on 8 trn2 cores.

Sharding: core = (batch, head_group): 4 batches x 2 head-groups of 8 heads.
Each core computes its batch's attention for its 8 heads plus the partial
output projection; the host sums the two head-group partials per batch and
adds the output bias.

Per-core kernel (fp32 PSUM accumulation throughout):
  x and Wq/Wk/Wv live in fp8e4m3 (weights pre-scaled 32x on the host to
  escape the e4m3 subnormal range); Q/K/V projections run fp8 DoubleRow
  matmuls (two k-tiles contracted per pass).  The 32*32 score scale is
  undone inside the softmax exp scale; the 32x on V cancels against a 32.0
  denominator-ones column.
  per head-pair m, query chunk c (512 wide), j-tile t:
    S^T pair: two row-tiled bf16 matmuls (contract rows 0-63 / 64-127)
              issued back-to-back -> run concurrently on the PE, into one
              [128, 2x512] PSUM tile
    expS = exp(S^T / (8*1024))  one ACT instruction for both heads, ->fp8
    causal mask on the diagonal block (one GPSIMD affine_select, both heads)
    per t-PAIR: ctxT_aug += V_aug^T-contract expS, a single fp8 DoubleRow
              matmul contracting both j-tiles (row 64 = denominator)
  ctx = ctxT[0:64] * (1/denom)          (lazy DVE/GPSIMD chain, see below)
  out_partial = ctx^T-contract Wo_s     [2048, 1024] in bf16

The scalar-engine exp stream is the attention-phase bottleneck, so the
tensor engine is kept busy through its exp waits: the S matmuls run one
iteration ahead of the PV matmuls, and projection work (V tiles, the next
pair's Q/K projection, the output projection) is drip-fed between S and PV
one matmul at a time from a chunk-filler queue.  The normalize chains are
emitted lazily, one stage per iteration, so their cross-engine waits never
head-of-line block an engine FIFO.
"""

import os
import sys

sys.path.insert(0, "/opt/trn_rl_repo")

import numpy as np

B = 4
T = 2048
D = 1024
H = 16
HD = 64
NCORES = 8
HPC = 8          # heads per core
DPC = HPC * HD   # 512
KT = D // 128    # 8 k-tiles
NT = T // 128    # 16 token tiles

_CACHE = {}
LAST_RESULTS = None


def _build_program():
    from contextlib import ExitStack

    import concourse.bass as bass
    import concourse.tile as tile
    from concourse import bacc, mybir

    f32 = mybir.dt.float32
    bf16 = mybir.dt.bfloat16
    f8 = mybir.dt.float8e4
    DR = mybir.MatmulPerfMode.DoubleRow
    Exp = mybir.ActivationFunctionType.Exp

    nc = bacc.Bacc(
        "TRN2", target_bir_lowering=False, debug=False, num_devices=NCORES
    )
    # x and the projection weights are fp8 (weights pre-scaled by 32 on the
    # host so their ~N(0, 0.02) values escape the e4m3 subnormal range); the
    # 32*32 scale on the scores is undone in the softmax exp scale, and the
    # 32x on V cancels against a 32.0 denominator-ones column
    xT = nc.dram_tensor("xT", [D, T], f8, kind="ExternalInput").ap()
    wq_d = nc.dram_tensor("wq", [D, DPC], f8, kind="ExternalInput").ap()
    wk_d = nc.dram_tensor("wk", [D, DPC], f8, kind="ExternalInput").ap()
    wv_d = nc.dram_tensor("wv", [D, DPC], f8, kind="ExternalInput").ap()
    wo_d = nc.dram_tensor("wo", [DPC, D], f8, kind="ExternalInput").ap()
    out_d = nc.dram_tensor("out", [T, D], f32, kind="ExternalOutput").ap()

    with tile.TileContext(nc) as tc, ExitStack() as top:
        persist = top.enter_context(tc.tile_pool(name="persist", bufs=1))
        xt_sb = persist.tile([128, KT, T], f8, tag="xt")
        qt = persist.tile([128, 4, T], bf16, tag="qt")
        kt = persist.tile([128, 4, T], bf16, tag="kt")
        # per-tile row stride 8*66=528 bytes keeps the t-pair step 16-aligned
        # for the DoubleRow PV weight AP; head h occupies cols 66h..66h+64
        v_sb = persist.tile([128, NT, HPC * (HD + 2)], f8, tag="v")
        ctx_sb = persist.tile([128, 4, T], f8, tag="ctx")
        wq = persist.tile([128, KT, DPC], f8, tag="wq")
        wk = persist.tile([128, KT, DPC], f8, tag="wk")
        wv = persist.tile([128, KT, DPC], f8, tag="wv")
        wo = persist.tile([128, 4, D], f8, tag="wo")

        # weight loads on the gpsimd queue so they overlap the x-chunk loads
        # on the sync/scalar queues; wv is split per k-pair so the first V
        # matmul starts as soon as the first quarter lands
        for kj in range(4):
            nc.gpsimd.dma_start(
                out=wv[:, 2 * kj : 2 * kj + 2, :],
                in_=wv_d.rearrange("(k p) d -> p k d", p=128)[
                    :, 2 * kj : 2 * kj + 2, :
                ],
            )
        for w_sb, w_d in ((wq, wq_d), (wk, wk_d)):
            nc.gpsimd.dma_start(
                out=w_sb, in_=w_d.rearrange("(k p) d -> p k d", p=128)
            )
        xT_r = xT.rearrange("(k p) t -> p k t", p=128)
        xq = (nc.sync, nc.scalar)
        for tci in range(8):
            xq[tci % 2].dma_start(
                out=xt_sb[:, :, 256 * tci : 256 * (tci + 1)],
                in_=xT_r[:, :, 256 * tci : 256 * (tci + 1)],
            )
        nc.gpsimd.dma_start(
            out=wo, in_=wo_d.rearrange("(c p) o -> p c o", p=128)
        )

        # denominator columns for the PV matmul; 32.0 cancels the 32x
        # weight pre-scale carried by V
        for h in range(HPC):
            nc.vector.memset(v_sb[:, :, 66 * h + 64 : 66 * h + 65], 32.0)

        expp = top.enter_context(tc.tile_pool(name="expp", bufs=4))
        smallp = top.enter_context(tc.tile_pool(name="smallp", bufs=2))
        outp = top.enter_context(tc.tile_pool(name="outp", bufs=3))
        # 8 PSUM banks: S pair tiles 2x2, ctx accumulators 2x1, filler 2x1.
        # The deferred normalize frees ctx slots within ~1 iteration, so the
        # ctx ring gets away with 2; the filler ring needs 2 so a chunk's
        # matmuls overlap the previous chunk's PSUM->SBUF copy.
        psS = top.enter_context(tc.tile_pool(name="psS", bufs=2, space="PSUM"))
        psC = top.enter_context(tc.tile_pool(name="psC", bufs=2, space="PSUM"))
        psF = top.enter_context(tc.tile_pool(name="psF", bufs=2, space="PSUM"))

        # ---------------- filler chunk builders ----------------
        # Each builder returns (matmul_thunks, finish_thunk); the filler
        # queue steps one matmul at a time between the S and PV matmuls of
        # the attention loop so the PE never idles on the exp stream.
        state = {"v_done": -1, "qk0_cnt": 0, "mul3_done": -1}

        def v_chunk(t):
            def build():
                ps = psF.tile([128, 512], f32, tag="fill", name=f"psv_{t}")
                mms = [
                    (lambda k=k: nc.tensor.matmul(
                        ps,
                        xt_sb[:, 2 * k : 2 * k + 2, 128 * t : 128 * (t + 1)],
                        wv[:, 2 * k : 2 * k + 2, :],
                        start=(k == 0),
                        stop=(k == KT // 2 - 1),
                        perf_mode=DR,
                    ))
                    for k in range(KT // 2)
                ]

                def fin():
                    nc.vector.tensor_copy(
                        v_sb[:, t, :].rearrange("p (h c) -> p h c", c=HD + 2)[
                            :, :, 0:HD
                        ],
                        ps.rearrange("p (h c) -> p h c", c=HD),
                    )
                    state["v_done"] = t

                return mms, fin

            return build

        def qk_chunk(m, w_sb, dest, ci, count=False):
            def build():
                ps = psF.tile([128, 512], f32, tag="fill", name=f"psqk_{m}_{ci}")
                mms = [
                    (lambda k=k: nc.tensor.matmul(
                        ps,
                        w_sb[:, 2 * k : 2 * k + 2, 128 * m : 128 * (m + 1)],
                        xt_sb[:, 2 * k : 2 * k + 2, 512 * ci : 512 * (ci + 1)],
                        start=(k == 0),
                        stop=(k == KT // 2 - 1),
                        perf_mode=DR,
                    ))
                    for k in range(KT // 2)
                ]

                def fin():
                    nc.vector.tensor_copy(
                        dest[:, m, 512 * ci : 512 * (ci + 1)], ps
                    )
                    if count:
                        state["qk0_cnt"] += 1

                return mms, fin

            return build

        def out_chunk(tt, oc):
            def build():
                ps = psF.tile([128, 512], f32, tag="fill", name=f"pso_{tt}_{oc}")
                mms = [
                    (lambda j=j: nc.tensor.matmul(
                        ps,
                        ctx_sb[:, 2 * j : 2 * j + 2, 128 * tt : 128 * (tt + 1)],
                        wo[:, 2 * j : 2 * j + 2, 512 * oc : 512 * (oc + 1)],
                        start=(j == 0),
                        stop=(j == 1),
                        perf_mode=DR,
                    ))
                    for j in range(2)
                ]

                def fin():
                    ot = outp.tile([128, 512], f32, tag="ot", name=f"ot_{tt}_{oc}")
                    nc.vector.tensor_scalar_mul(ot, ps, 1.0 / 32.0)
                    nc.sync.dma_start(
                        out=out_d[
                            128 * tt : 128 * (tt + 1), 512 * oc : 512 * (oc + 1)
                        ],
                        in_=ot,
                    )

                return mms, fin

            return build

        class Filler:
            """Queue of (gate, chunk-builder); step() emits one matmul."""

            def __init__(self, chunks):
                self.chunks = chunks
                self.ci = 0
                self.mi = 0
                self.cur = None
                self.emitted = 0
                self.total = sum(n for _, n, _ in chunks)

            def step(self, idx):
                if self.ci >= len(self.chunks):
                    return False
                gate, _, build = self.chunks[self.ci]
                if callable(gate):
                    if not gate():
                        return False
                elif idx < gate:
                    return False
                if self.cur is None:
                    self.cur = build()
                mms, fin = self.cur
                mms[self.mi]()
                self.mi += 1
                self.emitted += 1
                if self.mi == len(mms):
                    fin()
                    self.cur = None
                    self.mi = 0
                    self.ci += 1
                return True

            def pace(self, idx, n_iters):
                target = (self.total * (idx + 1) + n_iters - 1) // n_iters
                while self.emitted < target and self.step(idx):
                    pass

            def drain(self):
                while self.step(1 << 30):
                    pass

        # ---------------- normalize ----------------
        # The raw copy frees the ctx PSUM slot immediately; the remaining
        # stages (approx-reciprocal of the denominator row, gpsimd partition
        # broadcast, divide-multiply into ctx_sb) are pushed onto a lazy
        # queue and popped a couple of iterations later, when their inputs
        # have long completed -- an eagerly-emitted chain would head-of-line
        # block the DVE/gpsimd FIFOs behind its cross-engine waits.
        from collections import deque

        normq = deque()

        def normalize(ctx_ps, h, c):
            dq = h // 2
            pr = (h % 2) * 64
            raw = smallp.tile([65, 512], f32, tag="raw", bufs=4,
                              name=f"raw_{h}_{c}")
            nc.vector.tensor_copy(raw, ctx_ps)
            sp = smallp.tile([128, 4], f32, tag="sp", bufs=3,
                             name=f"sp_{h}_{c}")
            rp = smallp.tile([128, 4], f32, tag="rp", bufs=3,
                             name=f"rp_{h}_{c}")
            recip = smallp.tile([1, 512], f32, tag="recip", bufs=3,
                                name=f"rc_{h}_{c}")
            bc = smallp.tile([64, 512], f32, tag="bc", bufs=3,
                             name=f"bc_{h}_{c}")
            # the [1, 512] denominator row would cost ~8 cyc/elem on a single
            # DVE lane; spread it over 128 lanes with two tiny DMA reshapes
            normq.append(lambda: nc.sync.dma_start(out=sp, in_=raw[64:65, :]))
            normq.append(lambda: nc.vector.reciprocal(rp, sp))
            normq.append(lambda: nc.sync.dma_start(out=recip, in_=rp))

            normq.append(lambda: nc.gpsimd.partition_broadcast(bc, recip))

            def mul_stage():
                nc.vector.tensor_mul(
                    ctx_sb[pr : pr + 64, dq, 512 * c : 512 * (c + 1)],
                    raw[0:64, :],
                    bc,
                )
                if dq == 3 and pr == 64:
                    # both halves of pair-3 chunk c are now written; unblocks
                    # the out-projection filler chunks that read them
                    state["mul3_done"] = c

            normq.append(mul_stage)

        # ---------------- attention for one head pair ----------------
        def attention(m, filler):
            iters = [(c, t) for c in range(4) for t in range(4 * c + 4)]

            def emit_S(c, t):
                if m == 0:
                    # pair-0 Q/K projection chunks stream through the filler;
                    # chunk c of the t-loop needs projection chunks <= c
                    while 1 + state["qk0_cnt"] // 2 <= c and filler.step(1 << 30):
                        pass
                    assert 1 + state["qk0_cnt"] // 2 > c
                i0 = max(128 * t, 512 * c)
                ext = 512 * (c + 1) - i0
                sp = psS.tile([128, 1024], f32, tag="sps",
                              name=f"sps_{m}_{c}_{t}")
                for half in range(2):
                    pr = 64 * half
                    nc.tensor.matmul(
                        sp[:, 512 * half : 512 * half + ext],
                        kt[pr : pr + 64, m, 128 * t : 128 * (t + 1)],
                        qt[pr : pr + 64, m, i0 : i0 + ext],
                        start=True,
                        stop=True,
                    )
                return sp

            ctx_pair = [None, None]
            es4 = None
            base = 0
            sp_next = emit_S(*iters[0])
            for idx, (c, t) in enumerate(iters):
                sp = sp_next
                i0 = max(128 * t, 512 * c)
                ext = 512 * (c + 1) - i0
                if t % 2 == 0:
                    # one es tile per t-pair: [p, t-slot, head-half, 512];
                    # the PV matmul contracts both t-slots in one DoubleRow
                    # pass
                    es = expp.tile([128, 2048], f8, tag="es",
                                   name=f"es_{m}_{c}_{t}")
                    es4 = es.rearrange("p (u g x) -> p u g x", u=2, g=2)
                    base = i0
                    if t >= 4 * c:
                        # diagonal-region pair: the odd slot's leading 128
                        # columns are never written by its exp; zero them so
                        # the PV contraction ignores them
                        nc.vector.memset(es4[:, 1, :, 0:128], 0.0)
                off = i0 - base
                nc.scalar.activation(
                    es4[:, t % 2, :, off : off + ext],
                    sp.rearrange("p (g x) -> p g x", g=2)[:, :, 0:ext],
                    Exp,
                    scale=0.125 / 1024.0,
                )
                if i0 == 128 * t:
                    # keep element iff free_idx - partition_idx >= 0, applied
                    # to the diagonal 128-block of both heads in one call
                    nc.gpsimd.affine_select(
                        out=es4[:, t % 2, :, off : off + 128],
                        in_=es4[:, t % 2, :, off : off + 128],
                        compare_op=mybir.AluOpType.is_ge,
                        fill=0.0,
                        base=0,
                        pattern=[[0, 2], [1, 128]],
                        channel_multiplier=-1,
                    )
                # S one iteration ahead of PV, so the exp stream always has
                # material while the PE works through filler matmuls
                if idx + 1 < len(iters):
                    sp_next = emit_S(*iters[idx + 1])
                if m == 0:
                    while state["v_done"] < t and filler.step(idx):
                        pass
                    assert state["v_done"] >= t
                filler.pace(idx, len(iters))
                if normq:
                    normq.popleft()()
                if t % 2 == 1:
                    u = t // 2
                    if u == 0:
                        for half in range(2):
                            ctx_pair[half] = psC.tile(
                                [65, 512], f32, tag="ctxps",
                                name=f"ctxps_{m}_{c}_{half}",
                            )
                    ext0 = 512 * (c + 1) - base
                    for half in range(2):
                        h = 2 * m + half
                        nc.tensor.matmul(
                            ctx_pair[half][:, base - 512 * c : 512 * (c + 1) - 512 * c],
                            v_sb[:, t - 1 : t + 1, 66 * h : 66 * h + 65],
                            es4[:, :, half, 0:ext0],
                            start=(u == 0),
                            stop=(u == 2 * c + 1),
                            perf_mode=DR,
                        )
                if t == 4 * c + 3:
                    for half in range(2):
                        normalize(ctx_pair[half], 2 * m + half, c)

        # ---------------- emission schedule ----------------
        # upfront: only the first pair-0 Q/K projection chunk (the S matmuls
        # need it immediately); even the first V tiles drip-feed through the
        # t-loop, whose v_done forcing lands them just before their PVs
        f0 = Filler(
            [(0, KT // 2, qk_chunk(0, w, d, 0)) for w, d in ((wq, qt), (wk, kt))]
        )
        f0.drain()

        # iteration index at which query-chunk c of the t-loop is finished
        cend = [4, 12, 24, 40]
        fillers = [
            Filler(
                # ordered by consumption deadline in the pair-0 t-loop
                [(0, KT // 2, v_chunk(t)) for t in range(4)]
                + [(0, KT // 2, qk_chunk(0, w, d, 1, count=True))
                   for w, d in ((wq, qt), (wk, kt))]
                + [(0, KT // 2, v_chunk(t)) for t in range(4, 8)]
                + [(0, KT // 2, qk_chunk(0, w, d, 2, count=True))
                   for w, d in ((wq, qt), (wk, kt))]
                + [(0, KT // 2, v_chunk(t)) for t in range(8, 12)]
                + [(0, KT // 2, qk_chunk(0, w, d, 3, count=True))
                   for w, d in ((wq, qt), (wk, kt))]
                + [(0, KT // 2, v_chunk(t)) for t in range(12, NT)]
                + [(0, KT // 2, qk_chunk(1, w, d, ci))
                   for w, d in ((wq, qt), (wk, kt)) for ci in range(4)]
            ),
            Filler([(0, KT // 2, qk_chunk(2, w, d, ci))
                    for w, d in ((wq, qt), (wk, kt)) for ci in range(4)]
                   + [(0, KT // 2, qk_chunk(3, w, d, ci))
                      for w, d in ((wq, qt), (wk, kt)) for ci in range(2)]),
            Filler([(0, KT // 2, qk_chunk(3, w, d, ci))
                    for w, d in ((wq, qt), (wk, kt)) for ci in range(2, 4)]),
            # out-proj of token tile tt needs the pair-3 lazy normalize of
            # query chunk tt//4 to have drained from the queue
            Filler([((lambda cc=tt // 4: state["mul3_done"] >= cc), 2,
                     out_chunk(tt, oc))
                    for tt in range(12) for oc in range(2)]),
        ]
        for m in range(3):
            attention(m, fillers[m])
            fillers[m].drain()
        attention(3, fillers[3])
        # flush the remaining (pair-3 c=3) normalize stages before the PE
        # drain work so the reciprocal chain overlaps it
        while normq:
            normq.popleft()()
        fillers[3].drain()
        # tail: remaining out-proj tiles, double-wide on the freed psS ring
        # so consecutive chunks pipeline instead of serializing on the
        # single-buffer filler PSUM
        for tt in range(12, NT):
            ps = psS.tile([128, 1024], f32, tag="sps", name=f"psob_{tt}")
            for oc in range(2):
                for j in range(2):
                    nc.tensor.matmul(
                        ps[:, 512 * oc : 512 * (oc + 1)],
                        ctx_sb[:, 2 * j : 2 * j + 2, 128 * tt : 128 * (tt + 1)],
                        wo[:, 2 * j : 2 * j + 2, 512 * oc : 512 * (oc + 1)],
                        start=(j == 0),
                        stop=(j == 1),
                        perf_mode=DR,
                    )
            ot = outp.tile([128, 1024], f32, tag="otb", name=f"otb_{tt}")
            nc.vector.tensor_scalar_mul(ot, ps, 1.0 / 32.0)
            nc.sync.dma_start(out=out_d[128 * tt : 128 * (tt + 1), :], in_=ot)

    nc.compile()
    return nc


def _get_program():
    if "nc" not in _CACHE:
        _CACHE["nc"] = _build_program()
    return _CACHE["nc"]


def make_in_maps(x, Wq, Wk, Wv, Wo):
    import ml_dtypes

    bf16 = ml_dtypes.bfloat16
    f8 = ml_dtypes.float8_e4m3
    in_maps = []
    for core in range(NCORES):
        b, hg = core // 2, core % 2
        sl = slice(DPC * hg, DPC * (hg + 1))
        in_maps.append(
            {
                "xT": np.ascontiguousarray(x[b].T).astype(f8),
                # 32x pre-scale lifts the ~N(0, 0.02) weights out of the
                # fp8e4m3 subnormal range; undone on-device (see kernel doc)
                "wq": np.ascontiguousarray(32.0 * Wq[:, sl]).astype(f8),
                "wk": np.ascontiguousarray(32.0 * Wk[:, sl]).astype(f8),
                "wv": np.ascontiguousarray(32.0 * Wv[:, sl]).astype(f8),
                "wo": np.ascontiguousarray(32.0 * Wo[sl, :]).astype(f8),
            }
        )
    return in_maps


def kernel(x, Wq, Wk, Wv, Wo, bo):
    global LAST_RESULTS
    from concourse.bass_utils import run_bass_kernel_spmd

    x = np.asarray(x, dtype=np.float32)
    nc = _get_program()
    in_maps = make_in_maps(
        x,
        np.asarray(Wq, np.float32),
        np.asarray(Wk, np.float32),
        np.asarray(Wv, np.float32),
        np.asarray(Wo, np.float32),
    )
    res = run_bass_kernel_spmd(
        nc,
        in_maps,
        list(range(NCORES)),
        trace=bool(int(os.environ.get("KERNEL_TRACE", "0"))),
    )
    LAST_RESULTS = res
    bo = np.asarray(bo, np.float32)
    out = np.empty((B, T, D), np.float32)
    for b in range(B):
        out[b] = res.results[2 * b]["out"] + res.results[2 * b + 1]["out"] + bo
    return out
